# revision 10
# baseline (speedup 1.0000x reference)
"""Multi-head relative-position attention (T5-style) with LoRA on Q/V/O,
for Trainium2, distributed over 8 NeuronCores.

Sharding: core = (batch b = cid//2, query-half = cid%2). Each core computes
its 1024 query rows of the output completely (no cross-core reduction).
The position_bias output (head-indexed, batch-free) is sharded one head per
core; it is written on-device from a host-prepared Toeplitz sliding window
of the 4095-entry relative-bias row.

Attention math on device (per core):
  S^T[t, q] = (Wq' q)^T-projected  khT^T @ qhT  (K=64, two heads packed in
              the PE array via tile_position), 1/sqrt(d) folded into Wq.
  P = exp(S + bias) * mask, computed as
      exp(S + b_base) * (sfix * expwin) * mask
  where b_base is the far-field bucket constant (rel<=-128 or >=128 side),
  expwin = exp(bias - b31) are host-built sliding windows (Toeplitz), and
  sfix = exp(b31 - b_base). Softmax denominator comes free as a ones-column
  appended to V in the P^T @ V matmul. No row-max subtraction: |S| is
  bounded (~1.5) by construction, exp cannot overflow.
"""

import math
import sys

sys.path.insert(0, "/opt/trn_rl_repo")

import numpy as np

import concourse.bass as bass  # noqa: F401  (engine classes referenced via nc)
import concourse.mybir as mybir
import concourse.tile as tile
from concourse import bacc
from concourse.bass_utils import run_bass_kernel_spmd
from concourse.masks import make_identity

F32 = mybir.dt.float32
F16 = mybir.dt.float16
I32 = mybir.dt.int32
MUL = mybir.AluOpType.mult
EXP = mybir.ActivationFunctionType.Exp

B, TQ, TKV, D, H, DH = 4, 2048, 2048, 512, 8, 64
QH = TQ // 2          # query rows per core
NCORES = 8
NTT = TKV // 128      # 16 key tiles
NQT = QH // 128       # 8 query tiles per core

# window-multiply column ranges per key-tile tt (same for every core; the
# window *content* encodes the core's query offset). L covers the b15-far +
# near band needed by half-1 cores, R covers near + b15-far for half-0.
L_W = [min(max(128 * tt - 768, 0), QH) for tt in range(NTT)]
R_Q0 = [max(0, 128 * tt - 128) for tt in range(NTT)]
R_W = [(QH - R_Q0[tt]) if 128 * tt - 128 < QH else 0 for tt in range(NTT)]
L_FF = [1920 - 128 * tt for tt in range(NTT)]            # ff start in V2L
R_FF = [max(128 - 128 * tt, 0) for tt in range(NTT)]     # ff start in V2R
V2W = 1152                                               # window widths


def _bucket_table():
    """relative_position_bucket for rel = -2047..2047 (bidirectional).

    Computed with jax on its default backend so the bucket boundaries
    (float32 log + int32 cast) match the reference implementation run in
    the same environment bit-for-bit. Falls back to a float32 numpy
    replica if jax is unavailable."""
    try:
        import jax.numpy as jnp

        rel = jnp.arange(-2047, 2048, dtype=jnp.int32)
        nb = 16
        buckets = (rel > 0).astype(jnp.int32) * nb
        rp = jnp.abs(rel)
        max_exact = nb // 2
        is_small = rp < max_exact
        safe_rp = jnp.maximum(rp, 1)
        large = max_exact + (
            jnp.log(safe_rp.astype(jnp.float32) / max_exact)
            / math.log(128 / max_exact)
            * (nb - max_exact)
        ).astype(jnp.int32)
        large = jnp.minimum(large, nb - 1)
        return np.asarray(buckets + jnp.where(is_small, rp, large)).astype(
            np.int64)
    except Exception:
        rel = np.arange(-2047, 2048, dtype=np.int64)
        nb = 16
        buckets = (rel > 0).astype(np.int64) * nb
        rp = np.abs(rel)
        max_exact = nb // 2
        is_small = rp < max_exact
        safe_rp = np.maximum(rp, 1).astype(np.float32)
        large = max_exact + (
            np.log(safe_rp / np.float32(max_exact)).astype(np.float32)
            / np.float32(math.log(128 / max_exact))
            * np.float32(nb - max_exact)
        ).astype(np.int32)
        large = np.minimum(large, nb - 1)
        return (buckets + np.where(is_small, rp, large)).astype(np.int64)


_PROG_CACHE = {}


def _build_program():
    if "nc" in _PROG_CACHE:
        return _PROG_CACHE["nc"]
    nc = bacc.Bacc("TRN2", target_bir_lowering=False, debug=False,
                   num_devices=NCORES)

    d_in = {}

    def din(name, shape, dt):
        d_in[name] = nc.dram_tensor(name, shape, dt, kind="ExternalInput").ap()
        return d_in[name]

    qT = din("qT", [128, 4, QH], F32)        # [c%128, c//128, i]
    kT = din("kT", [128, 4, TKV], F32)
    vT = din("vT", [128, 4, TKV], F32)
    mk = din("mk", [QH, TKV], I32)           # mask natural [q, t]
    WqT = din("WqT", [128, 4, D], F32)       # (Wq/8).T   [c, hd]
    WkT = din("WkT", [128, 4, D], F32)
    WvT = din("WvT", [128, 4, D], F32)
    WoT = din("WoT", [128, 4, D], F32)       # Wo.T       [hd, o]
    AqT = din("AqT", [128, 4, 8], F32)       # Aq.T
    AvT = din("AvT", [128, 4, 8], F32)
    AoT = din("AoT", [128, 4, 8], F32)
    BqT = din("BqT", [8, D], F32)            # Bq.T / 64
    BvT = din("BvT", [8, D], F32)            # Bv.T / 8
    BoT = din("BoT", [8, D], F32)            # Bo.T / 8
    V2L = din("V2L", [128, H, V2W], F16)     # exp(bias - b31) windows
    V2R = din("V2R", [128, H, V2W], F16)
    bco = din("bco", [128, H], F32)          # exp bias constant (b_base)
    sfx = din("sfx", [128, H], F32)          # exp(b31 - b_base)
    W2O = din("W2O", [128, 3968], F32)       # position-bias out window

    out_o = nc.dram_tensor("out_o", [QH, D], F32, kind="ExternalOutput").ap()
    pb_o = nc.dram_tensor("pb_o", [TQ, TKV], F32, kind="ExternalOutput").ap()

    with tile.TileContext(nc) as tc:
        with tc.tile_pool(name="persist", bufs=1) as pers:

            # ---- phase 0: position_bias output (pure DMA) ----
            with tc.tile_pool(name="w2p", bufs=1) as w2p:
                w2 = w2p.tile([128, 3968], F32)
                nc.sync.dma_start(out=w2, in_=W2O)
                for a in range(16):
                    s0 = 1920 - 128 * a
                    nc.sync.dma_start(out=pb_o[a * 128:(a + 1) * 128, :],
                                      in_=w2[:, s0:s0 + TKV])

            # ---- persistent tiles ----
            mT = pers.tile([128, NTT, QH], F16)          # mask^T
            vh = pers.tile([128, NTT, H, 66], F16)       # V heads + ones col
            khT = pers.tile([128, 4, TKV], F32)
            qhT = pers.tile([128, 4, QH], F32)
            t_ones = pers.tile([1, 64], F32)

            nc.vector.memset(t_ones, 1.0)
            nc.vector.memset(vh[:, :, :, 64:65], 1.0)

            # ---- phase 1: mask convert + transpose ----
            with tc.tile_pool(name="mstage", bufs=2) as mst, \
                 tc.tile_pool(name="mps", bufs=2, space="PSUM") as mps:
                ident = pers.tile([128, 128], F16)
                make_identity(nc, ident)
                for qq in range(NQT):
                    mi = mst.tile([128, TKV], I32, tag="mi")
                    nc.sync.dma_start(out=mi, in_=mk[qq * 128:(qq + 1) * 128, :])
                    mf = mst.tile([128, TKV], F16, tag="mf")
                    nc.gpsimd.tensor_copy(mf, mi)
                    for hf in range(2):
                        pt = mps.tile([128, 1024], F16, tag="mtp")
                        for s in range(8):
                            ts_ = hf * 8 + s
                            nc.tensor.transpose(
                                pt[:, s * 128:(s + 1) * 128],
                                mf[:, ts_ * 128:(ts_ + 1) * 128], ident)
                        nc.vector.tensor_copy(
                            out=mT[:, hf * 8:(hf + 1) * 8,
                                   qq * 128:(qq + 1) * 128],
                            in_=pt.rearrange("p (s f) -> p s f", s=8))

            # ---- phase 2: projections ----
            # V (+ LoRA) -> vh natural [t, h, d] fp16, with ones column kept
            with tc.tile_pool(name="vstage", bufs=1) as vst, \
                 tc.tile_pool(name="vps1", bufs=1, space="PSUM") as vps1, \
                 tc.tile_pool(name="vps", bufs=2, space="PSUM") as vps:
                t_vT = vst.tile([128, 4, TKV], F32)
                nc.sync.dma_start(out=t_vT, in_=vT)
                t_WvT = vst.tile([128, 4, D], F32)
                nc.sync.dma_start(out=t_WvT, in_=WvT)
                t_AvT = vst.tile([128, 4, 8], F32)
                nc.sync.dma_start(out=t_AvT, in_=AvT)
                t_BvT = vst.tile([8, D], F32)
                nc.sync.dma_start(out=t_BvT, in_=BvT)
                tvp = vps1.tile([8, TKV], F32, tag="tvp")
                for cc in range(4):
                    for nn in range(4):
                        nc.tensor.matmul(tvp[:, nn * 512:(nn + 1) * 512],
                                         t_AvT[:, cc, :],
                                         t_vT[:, cc, nn * 512:(nn + 1) * 512],
                                         start=(cc == 0), stop=(cc == 3))
                t_tv = vst.tile([8, TKV], F32)
                nc.scalar.copy(t_tv, tvp)
                for tt in range(NTT):
                    vp = vps.tile([128, 512], F32, tag="vp")
                    for cc in range(4):
                        nc.tensor.matmul(vp, t_vT[:, cc, tt * 128:(tt + 1) * 128],
                                         t_WvT[:, cc, :],
                                         start=(cc == 0), stop=False)
                    nc.tensor.matmul(vp, t_tv[0:8, tt * 128:(tt + 1) * 128],
                                     t_BvT[0:8, :], start=False, stop=True)
                    nc.scalar.copy(vh[:, tt, :, 0:64],
                                   vp.rearrange("p (h d) -> p h d", h=H))

            # K -> khT [hd, t] fp32
            with tc.tile_pool(name="kstage", bufs=1) as kst, \
                 tc.tile_pool(name="kps", bufs=2, space="PSUM") as kps:
                t_kT = kst.tile([128, 4, TKV], F32)
                nc.sync.dma_start(out=t_kT, in_=kT)
                t_WkT = kst.tile([128, 4, D], F32)
                nc.sync.dma_start(out=t_WkT, in_=WkT)
                for hdt in range(4):
                    kp = kps.tile([128, TKV], F32, tag="kp")
                    for cc in range(4):
                        for nn in range(4):
                            nc.tensor.matmul(
                                kp[:, nn * 512:(nn + 1) * 512],
                                t_WkT[:, cc, hdt * 128:(hdt + 1) * 128],
                                t_kT[:, cc, nn * 512:(nn + 1) * 512],
                                start=(cc == 0), stop=(cc == 3))
                    nc.scalar.copy(khT[:, hdt, :], kp)

            # Q (+ LoRA, both 1/8 scales folded) -> qhT [hd, i] fp32
            with tc.tile_pool(name="qstage", bufs=1) as qst, \
                 tc.tile_pool(name="qps1", bufs=1, space="PSUM") as qps1, \
                 tc.tile_pool(name="qps", bufs=2, space="PSUM") as qps:
                t_qT = qst.tile([128, 4, QH], F32)
                nc.sync.dma_start(out=t_qT, in_=qT)
                t_WqT = qst.tile([128, 4, D], F32)
                nc.sync.dma_start(out=t_WqT, in_=WqT)
                t_AqT = qst.tile([128, 4, 8], F32)
                nc.sync.dma_start(out=t_AqT, in_=AqT)
                t_BqT = qst.tile([8, D], F32)
                nc.sync.dma_start(out=t_BqT, in_=BqT)
                tqp = qps1.tile([8, QH], F32, tag="tqp")
                for cc in range(4):
                    for nn in range(2):
                        nc.tensor.matmul(tqp[:, nn * 512:(nn + 1) * 512],
                                         t_AqT[:, cc, :],
                                         t_qT[:, cc, nn * 512:(nn + 1) * 512],
                                         start=(cc == 0), stop=(cc == 3))
                t_tq = qst.tile([8, QH], F32)
                nc.scalar.copy(t_tq, tqp)
                for hdt in range(4):
                    qp = qps.tile([128, QH], F32, tag="qp")
                    for nn in range(2):
                        for cc in range(4):
                            nc.tensor.matmul(
                                qp[:, nn * 512:(nn + 1) * 512],
                                t_WqT[:, cc, hdt * 128:(hdt + 1) * 128],
                                t_qT[:, cc, nn * 512:(nn + 1) * 512],
                                start=(cc == 0), stop=False)
                        nc.tensor.matmul(
                            qp[:, nn * 512:(nn + 1) * 512],
                            t_BqT[0:8, hdt * 128:(hdt + 1) * 128],
                            t_tq[0:8, nn * 512:(nn + 1) * 512],
                            start=False, stop=True)
                    nc.scalar.copy(qhT[:, hdt, :], qp)

            # ---- phase 3: attention, head pairs packed in the PE array ----
            with tc.tile_pool(name="pers2", bufs=1) as pers2, \
                 tc.tile_pool(name="p2dram", bufs=1, space="DRAM") as p2d:
                attnT = pers2.tile([128, 4, QH], F32)
                t_v2l = pers2.tile([128, H, V2W], F16)
                t_v2r = pers2.tile([128, H, V2W], F16)
                t_bco = pers2.tile([128, H], F32)
                t_sfx = pers2.tile([128, H], F32)
                den_d = p2d.tile([H, QH], F32)
                rec_d = p2d.tile([H, QH], F32)
                nc.sync.dma_start(out=t_v2l, in_=V2L)
                nc.sync.dma_start(out=t_v2r, in_=V2R)
                nc.sync.dma_start(out=t_bco, in_=bco)
                nc.sync.dma_start(out=t_sfx, in_=sfx)
                ctx3 = tc.tile_pool(name="apool", bufs=3)
                ap = ctx3.__enter__()
                ctx3b = tc.tile_pool(name="aps", bufs=1, space="PSUM")
                aps = ctx3b.__enter__()
                for i in range(4):
                    oA = aps.tile([65, QH], F32, tag="oA")
                    oB = aps.tile([65, QH], F32, tag="oB")
                    sA = aps.tile([128, QH], F32, tag="sA")
                    sB = aps.tile([128, QH], F32, tag="sB")
                    for tt in range(NTT):
                        ts_ = slice(tt * 128, (tt + 1) * 128)
                        for nn in range(2):
                            nsl = slice(nn * 512, (nn + 1) * 512)
                            nc.tensor.matmul(sA[:, nsl], khT[0:64, i, ts_],
                                             qhT[0:64, i, nsl],
                                             start=True, stop=True,
                                             tile_position=(0, 0))
                            nc.tensor.matmul(sB[:, nsl], khT[64:128, i, ts_],
                                             qhT[64:128, i, nsl],
                                             start=True, stop=True,
                                             tile_position=(64, 0))
                        for hh, S, O in ((2 * i, sA, oA), (2 * i + 1, sB, oB)):
                            PT = ap.tile([128, QH], F16, tag="PT")
                            nc.scalar.activation(PT, S, EXP,
                                                 bias=t_bco[:, hh:hh + 1],
                                                 scale=1.0)
                            if L_W[tt]:
                                nc.vector.scalar_tensor_tensor(
                                    out=PT[:, 0:L_W[tt]],
                                    in0=PT[:, 0:L_W[tt]],
                                    scalar=t_sfx[:, hh:hh + 1],
                                    in1=t_v2l[:, hh, L_FF[tt]:L_FF[tt] + L_W[tt]],
                                    op0=MUL, op1=MUL)
                            if R_W[tt]:
                                nc.vector.scalar_tensor_tensor(
                                    out=PT[:, R_Q0[tt]:R_Q0[tt] + R_W[tt]],
                                    in0=PT[:, R_Q0[tt]:R_Q0[tt] + R_W[tt]],
                                    scalar=t_sfx[:, hh:hh + 1],
                                    in1=t_v2r[:, hh, R_FF[tt]:R_FF[tt] + R_W[tt]],
                                    op0=MUL, op1=MUL)
                            # mask multiply: split between DVE and GPSIMD
                            if tt % 4 == 0:
                                nc.vector.tensor_tensor(out=PT, in0=PT,
                                                        in1=mT[:, tt, :], op=MUL)
                            else:
                                nc.gpsimd.tensor_tensor(out=PT, in0=PT,
                                                        in1=mT[:, tt, :], op=MUL)
                            for nn in range(2):
                                nsl = slice(nn * 512, (nn + 1) * 512)
                                nc.tensor.matmul(O[0:65, nsl],
                                                 vh[:, tt, hh, 0:65], PT[:, nsl],
                                                 start=(tt == 0), stop=(tt == 15))
                    for hh, O in ((2 * i, oA), (2 * i + 1, oB)):
                        p0 = (hh % 2) * 64
                        nc.vector.tensor_copy(attnT[p0:p0 + 64, i, :], O[0:64, :])
                        dtmp = ap.tile([1, QH], F32, tag="dtmp")
                        nc.vector.tensor_copy(dtmp, O[64:65, :])
                        nc.sync.dma_start(out=den_d[hh, :], in_=dtmp)

                ctx3b.__exit__(None, None, None)
                ctx3.__exit__(None, None, None)
                # ---- phase 4: softmax division ----
                with tc.tile_pool(name="dpool", bufs=2) as dp, \
                     tc.tile_pool(name="dps", bufs=2, space="PSUM") as dps:
                    dsq = dp.tile([128, H * QH // 128], F32, tag="dsq")
                    nc.sync.dma_start(
                        out=dsq,
                        in_=den_d.rearrange("h (p f) -> (h p) f", p=16))
                    rsq = dp.tile([128, H * QH // 128], F32, tag="rsq")
                    nc.vector.reciprocal(rsq, dsq)
                    nc.sync.dma_start(
                        out=rec_d.rearrange("h (p f) -> (h p) f", p=16),
                        in_=rsq)
                    for i in range(4):
                        rp = dps.tile([128, QH], F32, tag="rp")
                        rtmp0 = dp.tile([1, QH], F32, tag="rtmp0")
                        rtmp1 = dp.tile([1, QH], F32, tag="rtmp1")
                        rtmps = [rtmp0, rtmp1]
                        nc.sync.dma_start(out=rtmps[0], in_=rec_d[2 * i, :])
                        nc.sync.dma_start(out=rtmps[1], in_=rec_d[2 * i + 1, :])
                        for h2 in range(2):
                            for nn in range(2):
                                nc.tensor.matmul(
                                    rp[h2 * 64:(h2 + 1) * 64,
                                       nn * 512:(nn + 1) * 512],
                                    t_ones,
                                    rtmps[h2][0:1,
                                              nn * 512:(nn + 1) * 512],
                                    start=True, stop=True)
                        nc.vector.tensor_tensor(out=attnT[:, i, :],
                                                in0=attnT[:, i, :], in1=rp,
                                                op=MUL)

                # ---- phase 5: output projection (+ LoRA) ----
                with tc.tile_pool(name="opool", bufs=2) as op, \
                     tc.tile_pool(name="ofix", bufs=1) as ofix, \
                     tc.tile_pool(name="ops1", bufs=1, space="PSUM") as ops1, \
                     tc.tile_pool(name="ops", bufs=2, space="PSUM") as ops:
                    t_WoT = ofix.tile([128, 4, D], F32)
                    t_AoT = ofix.tile([128, 4, 8], F32)
                    t_BoT = ofix.tile([8, D], F32)
                    nc.sync.dma_start(out=t_WoT, in_=WoT)
                    nc.sync.dma_start(out=t_AoT, in_=AoT)
                    nc.sync.dma_start(out=t_BoT, in_=BoT)
                    top = ops1.tile([8, QH], F32, tag="top")
                    for hdt in range(4):
                        for nn in range(2):
                            nc.tensor.matmul(
                                top[:, nn * 512:(nn + 1) * 512],
                                t_AoT[:, hdt, :],
                                attnT[:, hdt, nn * 512:(nn + 1) * 512],
                                start=(hdt == 0), stop=(hdt == 3))
                    t_to = op.tile([8, QH], F32, tag="t_to")
                    nc.scalar.copy(t_to, top)
                    for qt in range(NQT):
                        qsl = slice(qt * 128, (qt + 1) * 128)
                        po = ops.tile([128, D], F32, tag="po")
                        for hdt in range(4):
                            nc.tensor.matmul(po, attnT[:, hdt, qsl],
                                             t_WoT[:, hdt, :],
                                             start=(hdt == 0), stop=False)
                        nc.tensor.matmul(po, t_to[0:8, qsl], t_BoT[0:8, :],
                                         start=False, stop=True)
                        ob = op.tile([128, D], F32, tag="ob")
                        nc.vector.tensor_copy(ob, po)
                        nc.sync.dma_start(out=out_o[qsl, :], in_=ob)

    nc.compile()
    _PROG_CACHE["nc"] = nc
    return nc


def _host_prep(inputs):
    """Build the 8 per-core input maps."""
    q, k, v, mask = inputs["q"], inputs["k"], inputs["v"], inputs["mask"]
    rel_emb = inputs["rel_emb"]

    table = _bucket_table()                        # [4095]
    biasrow = rel_emb[table, :]                    # [4095, H] fp32, exact
    b31 = rel_emb[31, :].astype(np.float64)        # far const, rel >= 128
    b15 = rel_emb[15, :].astype(np.float64)        # far const, rel <= -128

    def cm(x):  # [T, 512] -> [128, 4, T] channel-major contiguous
        return np.ascontiguousarray(
            x.T.reshape(4, 128, -1).transpose(1, 0, 2))

    def wprep(W, s=1.0):
        Wt = np.ascontiguousarray((W * s).T.astype(np.float32))  # [c(in), out]
        return np.ascontiguousarray(Wt.reshape(4, 128, 512).transpose(1, 0, 2))

    WqT = wprep(inputs["Wq"], 1.0 / 8.0)
    WkT = wprep(inputs["Wk"])
    WvT = wprep(inputs["Wv"])
    WoT = wprep(inputs["Wo"])

    def aprep(A):  # [8, 512] -> A.T [512, 8] -> [128, 4, 8]
        At = np.ascontiguousarray(A.T.astype(np.float32))
        return np.ascontiguousarray(At.reshape(4, 128, 8).transpose(1, 0, 2))

    AqT = aprep(inputs["Aq"])
    AvT = aprep(inputs["Av"])
    AoT = aprep(inputs["Ao"])
    BqT = np.ascontiguousarray(inputs["Bq"].T.astype(np.float32)) / 64.0
    BvT = np.ascontiguousarray(inputs["Bv"].T.astype(np.float32)) / 8.0
    BoT = np.ascontiguousarray(inputs["Bo"].T.astype(np.float32)) / 8.0

    p = np.arange(128)[:, None]
    in_maps = []
    for cid in range(NCORES):
        b, half = cid // 2, cid % 2
        r0 = half * QH

        # sliding windows: idx = p - f' + 4095 - r0 into biasrow
        ffl = np.arange(V2W)[None, :] + 128          # f' = ffL + 128
        ffr = np.arange(V2W)[None, :] + 1920         # f' = ffR + 1920
        idxl = np.clip(p - ffl + 4095 - r0, 0, 4094)
        idxr = np.clip(p - ffr + 4095 - r0, 0, 4094)
        bb = b31 if half == 0 else b15               # [H]
        V2La = np.exp(biasrow[idxl, :].astype(np.float64)
                      - b31[None, None, :]).astype(np.float16)
        V2Ra = np.exp(biasrow[idxr, :].astype(np.float64)
                      - b31[None, None, :]).astype(np.float16)
        # -> [128, H, V2W]
        V2La = np.ascontiguousarray(V2La.transpose(0, 2, 1))
        V2Ra = np.ascontiguousarray(V2Ra.transpose(0, 2, 1))
        bcoa = np.ascontiguousarray(
            np.broadcast_to(bb.astype(np.float32)[None, :], (128, H)))
        sfxa = np.ascontiguousarray(np.broadcast_to(
            np.exp(b31 - bb).astype(np.float32)[None, :], (128, H)))

        # position-bias window for head cid
        ffw = np.arange(3968)[None, :]
        W2Oa = np.ascontiguousarray(
            biasrow[np.clip(ffw - p + 127, 0, 4094), cid].astype(np.float32))

        in_maps.append({
            "qT": cm(q[b, r0:r0 + QH, :]),
            "kT": cm(k[b]),
            "vT": cm(v[b]),
            "mk": np.ascontiguousarray(mask[b, r0:r0 + QH, :]),
            "WqT": WqT, "WkT": WkT, "WvT": WvT, "WoT": WoT,
            "AqT": AqT, "AvT": AvT, "AoT": AoT,
            "BqT": BqT, "BvT": BvT, "BoT": BoT,
            "V2L": V2La, "V2R": V2Ra, "bco": bcoa, "sfx": sfxa,
            "W2O": W2Oa,
        })
    return in_maps


def kernel(**inputs):
    nc = _build_program()
    in_maps = _host_prep(inputs)
    res = run_bass_kernel_spmd(nc, in_maps, list(range(NCORES)))

    out = np.empty((B, TQ, D), np.float32)
    pb = np.empty((H, 1, TQ, TKV), np.float32)
    for cid in range(NCORES):
        b, half = cid // 2, cid % 2
        r0 = half * QH
        out[b, r0:r0 + QH, :] = res.results[cid]["out_o"]
        pb[cid, 0] = res.results[cid]["pb_o"]
    return out, pb


# revision 13
# speedup vs baseline: 1.1482x; 1.1482x over previous
"""Multi-head relative-position attention (T5-style) with LoRA on Q/V/O,
for Trainium2, distributed over 8 NeuronCores.

Sharding: core = (batch b = cid//2, query-half = cid%2). Each core computes
its 1024 query rows of the output completely (no cross-core reduction).
The position_bias output (head-indexed, batch-free) is sharded one head per
core; it is written on-device from a host-prepared Toeplitz sliding window
of the 4095-entry relative-bias row.

Attention math on device (per core):
  S^T[t, q] = (Wq' q)^T-projected  khT^T @ qhT  (K=64, two heads packed in
              the PE array via tile_position), 1/sqrt(d) folded into Wq.
  P = exp(S + bias) * mask, computed as
      exp(S + b_base) * (sfix * expwin) * mask
  where b_base is the far-field bucket constant (rel<=-128 or >=128 side),
  expwin = exp(bias - b31) are host-built sliding windows (Toeplitz), and
  sfix = exp(b31 - b_base). Softmax denominator comes free as a ones-column
  appended to V in the P^T @ V matmul. No row-max subtraction: |S| is
  bounded (~1.5) by construction, exp cannot overflow.
"""

import math
import sys

sys.path.insert(0, "/opt/trn_rl_repo")

import numpy as np

import concourse.bass as bass  # noqa: F401  (engine classes referenced via nc)
import concourse.mybir as mybir
import concourse.tile as tile
from concourse import bacc
from concourse.bass_utils import run_bass_kernel_spmd
from concourse.masks import make_identity

F32 = mybir.dt.float32
F32R = mybir.dt.float32r
F16 = mybir.dt.float16


I32 = mybir.dt.int32
MUL = mybir.AluOpType.mult
EXP = mybir.ActivationFunctionType.Exp

B, TQ, TKV, D, H, DH = 4, 2048, 2048, 512, 8, 64
QH = TQ // 2          # query rows per core
NCORES = 8
NTT = TKV // 128      # 16 key tiles
NQT = QH // 128       # 8 query tiles per core

# window-multiply column ranges per key-tile tt (same for every core; the
# window *content* encodes the core's query offset). L covers the b15-far +
# near band needed by half-1 cores, R covers near + b15-far for half-0.
L_W = [min(max(128 * tt - 768, 0), QH) for tt in range(NTT)]
R_Q0 = [max(0, 128 * tt - 128) for tt in range(NTT)]
R_W = [(QH - R_Q0[tt]) if 128 * tt - 128 < QH else 0 for tt in range(NTT)]
L_FF = [1920 - 128 * tt for tt in range(NTT)]            # ff start in V2L
R_FF = [max(128 - 128 * tt, 0) for tt in range(NTT)]     # ff start in V2R
V2W = 1152                                               # window widths


def _bucket_table():
    """relative_position_bucket for rel = -2047..2047 (bidirectional).

    Computed with jax on its default backend so the bucket boundaries
    (float32 log + int32 cast) match the reference implementation run in
    the same environment bit-for-bit. Falls back to a float32 numpy
    replica if jax is unavailable."""
    try:
        import jax.numpy as jnp

        rel = jnp.arange(-2047, 2048, dtype=jnp.int32)
        nb = 16
        buckets = (rel > 0).astype(jnp.int32) * nb
        rp = jnp.abs(rel)
        max_exact = nb // 2
        is_small = rp < max_exact
        safe_rp = jnp.maximum(rp, 1)
        large = max_exact + (
            jnp.log(safe_rp.astype(jnp.float32) / max_exact)
            / math.log(128 / max_exact)
            * (nb - max_exact)
        ).astype(jnp.int32)
        large = jnp.minimum(large, nb - 1)
        return np.asarray(buckets + jnp.where(is_small, rp, large)).astype(
            np.int64)
    except Exception:
        rel = np.arange(-2047, 2048, dtype=np.int64)
        nb = 16
        buckets = (rel > 0).astype(np.int64) * nb
        rp = np.abs(rel)
        max_exact = nb // 2
        is_small = rp < max_exact
        safe_rp = np.maximum(rp, 1).astype(np.float32)
        large = max_exact + (
            np.log(safe_rp / np.float32(max_exact)).astype(np.float32)
            / np.float32(math.log(128 / max_exact))
            * np.float32(nb - max_exact)
        ).astype(np.int32)
        large = np.minimum(large, nb - 1)
        return (buckets + np.where(is_small, rp, large)).astype(np.int64)


_PROG_CACHE = {}


def _build_program():
    if "nc" in _PROG_CACHE:
        return _PROG_CACHE["nc"]
    nc = bacc.Bacc("TRN2", target_bir_lowering=False, debug=False,
                   num_devices=NCORES)

    d_in = {}

    def din(name, shape, dt):
        d_in[name] = nc.dram_tensor(name, shape, dt, kind="ExternalInput").ap()
        return d_in[name]

    qT = din("qT", [128, 4, QH], F32R)        # [c%128, c//128, i]
    kT = din("kT", [128, 4, TKV], F32R)
    vT = din("vT", [128, 4, TKV], F32R)
    mk = din("mk", [QH, TKV], I32)           # mask natural [q, t]
    WqT = din("WqT", [128, 4, D], F32R)       # (Wq/8).T   [c, hd]
    WkT = din("WkT", [128, 4, D], F32R)
    WvT = din("WvT", [128, 4, D], F32R)
    WoT = din("WoT", [128, 4, D], F32R)       # Wo.T       [hd, o]
    AqT = din("AqT", [128, 4, 8], F32R)       # Aq.T
    AvT = din("AvT", [128, 4, 8], F32R)
    AoT = din("AoT", [128, 4, 8], F32R)
    BqT = din("BqT", [8, D], F32R)            # Bq.T / 64
    BvT = din("BvT", [8, D], F32R)            # Bv.T / 8
    BoT = din("BoT", [8, D], F32R)            # Bo.T / 8
    V2L = din("V2L", [128, H, V2W], F16)     # exp(bias - b31) windows
    V2R = din("V2R", [128, H, V2W], F16)
    bco = din("bco", [128, H], F32)          # exp bias constant (b_base)
    sfx = din("sfx", [128, H], F32)          # exp(b31 - b_base)
    W2O = din("W2O", [128, 3968], F32)       # position-bias out window

    out_o = nc.dram_tensor("out_o", [QH, D], F32, kind="ExternalOutput").ap()
    pb_o = nc.dram_tensor("pb_o", [TQ, TKV], F32, kind="ExternalOutput").ap()

    with tile.TileContext(nc) as tc:
        with tc.tile_pool(name="persist", bufs=1) as pers:

            # ---- phase 0: position_bias output (pure DMA) ----
            with tc.tile_pool(name="w2p", bufs=1) as w2p:
                w2 = w2p.tile([128, 3968], F32)
                nc.sync.dma_start(out=w2, in_=W2O)
                for a in range(16):
                    s0 = 1920 - 128 * a
                    nc.sync.dma_start(out=pb_o[a * 128:(a + 1) * 128, :],
                                      in_=w2[:, s0:s0 + TKV])

            # ---- persistent tiles ----
            mT = pers.tile([128, NTT, QH], F16)          # mask^T
            vh = pers.tile([128, NTT, H, 66], F16)       # V heads + ones col
            khT = pers.tile([128, 4, TKV], F32R)
            qhT = pers.tile([128, 4, QH], F32R)
            t_ones = pers.tile([1, 64], F32)

            nc.vector.memset(t_ones, 1.0)
            nc.vector.memset(vh[:, :, :, 64:65], 1.0)

            # ---- phase 1: mask convert + transpose ----
            with tc.tile_pool(name="mstage", bufs=2) as mst, \
                 tc.tile_pool(name="mps", bufs=2, space="PSUM") as mps:
                ident = pers.tile([128, 128], F16)
                make_identity(nc, ident)
                for qq in range(NQT):
                    mi = mst.tile([128, TKV], I32, tag="mi")
                    nc.sync.dma_start(out=mi, in_=mk[qq * 128:(qq + 1) * 128, :])
                    mf = mst.tile([128, TKV], F16, tag="mf")
                    nc.gpsimd.tensor_copy(mf, mi)
                    for hf in range(2):
                        pt = mps.tile([128, 1024], F16, tag="mtp")
                        for s in range(8):
                            ts_ = hf * 8 + s
                            nc.tensor.transpose(
                                pt[:, s * 128:(s + 1) * 128],
                                mf[:, ts_ * 128:(ts_ + 1) * 128], ident)
                        nc.vector.tensor_copy(
                            out=mT[:, hf * 8:(hf + 1) * 8,
                                   qq * 128:(qq + 1) * 128],
                            in_=pt.rearrange("p (s f) -> p s f", s=8))

            # ---- phase 2: projections ----
            # V (+ LoRA) -> vh natural [t, h, d] fp16, with ones column kept
            with tc.tile_pool(name="vstage", bufs=1) as vst, \
                 tc.tile_pool(name="vps1", bufs=1, space="PSUM") as vps1, \
                 tc.tile_pool(name="vps", bufs=2, space="PSUM") as vps:
                t_vT = vst.tile([128, 4, TKV], F32R)
                nc.sync.dma_start(out=t_vT, in_=vT)
                t_WvT = vst.tile([128, 4, D], F32R)
                nc.sync.dma_start(out=t_WvT, in_=WvT)
                t_AvT = vst.tile([128, 4, 8], F32R)
                nc.sync.dma_start(out=t_AvT, in_=AvT)
                t_BvT = vst.tile([8, D], F32R)
                nc.sync.dma_start(out=t_BvT, in_=BvT)
                tvp = vps1.tile([8, TKV], F32, tag="tvp")
                for cc in range(4):
                    for nn in range(4):
                        nc.tensor.matmul(tvp[:, nn * 512:(nn + 1) * 512],
                                         t_AvT[:, cc, :],
                                         t_vT[:, cc, nn * 512:(nn + 1) * 512],
                                         start=(cc == 0), stop=(cc == 3))
                t_tv = vst.tile([8, TKV], F32R)
                nc.scalar.copy(t_tv, tvp)
                for tt in range(NTT):
                    vp = vps.tile([128, 512], F32, tag="vp")
                    for cc in range(4):
                        nc.tensor.matmul(vp,
                                         t_vT[:, cc, tt * 128:(tt + 1) * 128],
                                         t_WvT[:, cc, :],
                                         start=(cc == 0), stop=False)
                    nc.tensor.matmul(vp, t_tv[0:8, tt * 128:(tt + 1) * 128],
                                     t_BvT[0:8, :], start=False, stop=True)
                    nc.scalar.copy(vh[:, tt, :, 0:64],
                                   vp.rearrange("p (h d) -> p h d", h=H))

            # K -> khT [hd, t] fp32
            with tc.tile_pool(name="kstage", bufs=1) as kst, \
                 tc.tile_pool(name="kps", bufs=2, space="PSUM") as kps:
                t_kT = kst.tile([128, 4, TKV], F32R)
                nc.sync.dma_start(out=t_kT, in_=kT)
                t_WkT = kst.tile([128, 4, D], F32R)
                nc.sync.dma_start(out=t_WkT, in_=WkT)
                for hdt in range(4):
                    kp = kps.tile([128, TKV], F32, tag="kp")
                    for cc in range(4):
                        for nn in range(4):
                            nc.tensor.matmul(
                                kp[:, nn * 512:(nn + 1) * 512],
                                t_WkT[:, cc, hdt * 128:(hdt + 1) * 128],
                                t_kT[:, cc, nn * 512:(nn + 1) * 512],
                                start=(cc == 0), stop=(cc == 3))
                    nc.scalar.copy(khT[:, hdt, :], kp)

            # Q (+ LoRA, both 1/8 scales folded) -> qhT [hd, i] fp32
            with tc.tile_pool(name="qstage", bufs=1) as qst, \
                 tc.tile_pool(name="qps1", bufs=1, space="PSUM") as qps1, \
                 tc.tile_pool(name="qps", bufs=2, space="PSUM") as qps:
                t_qT = qst.tile([128, 4, QH], F32R)
                nc.sync.dma_start(out=t_qT, in_=qT)
                t_WqT = qst.tile([128, 4, D], F32R)
                nc.sync.dma_start(out=t_WqT, in_=WqT)
                t_AqT = qst.tile([128, 4, 8], F32R)
                nc.sync.dma_start(out=t_AqT, in_=AqT)
                t_BqT = qst.tile([8, D], F32R)
                nc.sync.dma_start(out=t_BqT, in_=BqT)
                tqp = qps1.tile([8, QH], F32, tag="tqp")
                for cc in range(4):
                    for nn in range(2):
                        nc.tensor.matmul(tqp[:, nn * 512:(nn + 1) * 512],
                                         t_AqT[:, cc, :],
                                         t_qT[:, cc, nn * 512:(nn + 1) * 512],
                                         start=(cc == 0), stop=(cc == 3))
                t_tq = qst.tile([8, QH], F32R)
                nc.scalar.copy(t_tq, tqp)
                for hdt in range(4):
                    qp = qps.tile([128, QH], F32, tag="qp")
                    for nn in range(2):
                        for cc in range(4):
                            nc.tensor.matmul(
                                qp[:, nn * 512:(nn + 1) * 512],
                                t_WqT[:, cc, hdt * 128:(hdt + 1) * 128],
                                t_qT[:, cc, nn * 512:(nn + 1) * 512],
                                start=(cc == 0), stop=False)
                        nc.tensor.matmul(
                            qp[:, nn * 512:(nn + 1) * 512],
                            t_BqT[0:8, hdt * 128:(hdt + 1) * 128],
                            t_tq[0:8, nn * 512:(nn + 1) * 512],
                            start=False, stop=True)
                    nc.scalar.copy(qhT[:, hdt, :], qp)

            # ---- phase 3: attention, head pairs packed in the PE array ----
            with tc.tile_pool(name="pers2", bufs=1) as pers2, \
                 tc.tile_pool(name="p2dram", bufs=1, space="DRAM") as p2d:
                attnT = pers2.tile([128, 4, QH], F32R)
                t_v2l = pers2.tile([128, H, V2W], F16)
                t_v2r = pers2.tile([128, H, V2W], F16)
                t_bco = pers2.tile([128, H], F32)
                t_sfx = pers2.tile([128, H], F32)
                den_d = p2d.tile([H, QH], F32)
                rec_d = p2d.tile([H, QH], F32)
                nc.sync.dma_start(out=t_v2l, in_=V2L)
                nc.sync.dma_start(out=t_v2r, in_=V2R)
                nc.sync.dma_start(out=t_bco, in_=bco)
                nc.sync.dma_start(out=t_sfx, in_=sfx)
                ctx3 = tc.tile_pool(name="apool", bufs=3)
                ap = ctx3.__enter__()
                ctx3b = tc.tile_pool(name="aps", bufs=1, space="PSUM")
                aps = ctx3b.__enter__()
                for i in range(4):
                    oA = aps.tile([65, QH], F32, tag="oA")
                    oB = aps.tile([65, QH], F32, tag="oB")
                    sA = aps.tile([128, QH], F32, tag="sA")
                    sB = aps.tile([128, QH], F32, tag="sB")
                    for tt in range(NTT):
                        ts_ = slice(tt * 128, (tt + 1) * 128)
                        for nn in range(2):
                            nsl = slice(nn * 512, (nn + 1) * 512)
                            nc.tensor.matmul(sA[:, nsl], khT[0:64, i, ts_],
                                             qhT[0:64, i, nsl],
                                             start=True, stop=True,
                                             tile_position=(0, 0))
                            nc.tensor.matmul(sB[:, nsl], khT[64:128, i, ts_],
                                             qhT[64:128, i, nsl],
                                             start=True, stop=True,
                                             tile_position=(64, 0))
                        for hh, S, O in ((2 * i, sA, oA), (2 * i + 1, sB, oB)):
                            PT = ap.tile([128, QH], F16, tag="PT")
                            nc.scalar.activation(PT, S, EXP,
                                                 bias=t_bco[:, hh:hh + 1],
                                                 scale=1.0)
                            if L_W[tt]:
                                nc.vector.scalar_tensor_tensor(
                                    out=PT[:, 0:L_W[tt]],
                                    in0=PT[:, 0:L_W[tt]],
                                    scalar=t_sfx[:, hh:hh + 1],
                                    in1=t_v2l[:, hh, L_FF[tt]:L_FF[tt] + L_W[tt]],
                                    op0=MUL, op1=MUL)
                            if R_W[tt]:
                                nc.vector.scalar_tensor_tensor(
                                    out=PT[:, R_Q0[tt]:R_Q0[tt] + R_W[tt]],
                                    in0=PT[:, R_Q0[tt]:R_Q0[tt] + R_W[tt]],
                                    scalar=t_sfx[:, hh:hh + 1],
                                    in1=t_v2r[:, hh, R_FF[tt]:R_FF[tt] + R_W[tt]],
                                    op0=MUL, op1=MUL)
                            nc.vector.tensor_tensor(out=PT, in0=PT,
                                                    in1=mT[:, tt, :], op=MUL)
                            for nn in range(2):
                                nsl = slice(nn * 512, (nn + 1) * 512)
                                nc.tensor.matmul(O[0:65, nsl],
                                                 vh[:, tt, hh, 0:65], PT[:, nsl],
                                                 start=(tt == 0), stop=(tt == 15))
                    for hh, O in ((2 * i, oA), (2 * i + 1, oB)):
                        p0 = (hh % 2) * 64
                        nc.vector.tensor_copy(attnT[p0:p0 + 64, i, :], O[0:64, :])
                        dtmp = ap.tile([1, QH], F32, tag="dtmp")
                        nc.vector.tensor_copy(dtmp, O[64:65, :])
                        nc.sync.dma_start(out=den_d[hh, :], in_=dtmp)

                ctx3b.__exit__(None, None, None)
                ctx3.__exit__(None, None, None)
                # ---- phase 4: softmax division ----
                with tc.tile_pool(name="dpool", bufs=2) as dp, \
                     tc.tile_pool(name="dps", bufs=2, space="PSUM") as dps:
                    dsq = dp.tile([128, H * QH // 128], F32, tag="dsq")
                    nc.sync.dma_start(
                        out=dsq,
                        in_=den_d.rearrange("h (p f) -> (h p) f", p=16))
                    rsq = dp.tile([128, H * QH // 128], F32, tag="rsq")
                    nc.vector.reciprocal(rsq, dsq)
                    nc.sync.dma_start(
                        out=rec_d.rearrange("h (p f) -> (h p) f", p=16),
                        in_=rsq)
                    for i in range(4):
                        rp = dps.tile([128, QH], F32, tag="rp")
                        rtmp0 = dp.tile([1, QH], F32, tag="rtmp0")
                        rtmp1 = dp.tile([1, QH], F32, tag="rtmp1")
                        rtmps = [rtmp0, rtmp1]
                        nc.sync.dma_start(out=rtmps[0], in_=rec_d[2 * i, :])
                        nc.sync.dma_start(out=rtmps[1], in_=rec_d[2 * i + 1, :])
                        for h2 in range(2):
                            for nn in range(2):
                                nc.tensor.matmul(
                                    rp[h2 * 64:(h2 + 1) * 64,
                                       nn * 512:(nn + 1) * 512],
                                    t_ones,
                                    rtmps[h2][0:1,
                                              nn * 512:(nn + 1) * 512],
                                    start=True, stop=True)
                        nc.vector.tensor_tensor(out=attnT[:, i, :],
                                                in0=attnT[:, i, :], in1=rp,
                                                op=MUL)

                # ---- phase 5: output projection (+ LoRA) ----
                with tc.tile_pool(name="opool", bufs=2) as op, \
                     tc.tile_pool(name="ofix", bufs=1) as ofix, \
                     tc.tile_pool(name="ops1", bufs=1, space="PSUM") as ops1, \
                     tc.tile_pool(name="ops", bufs=2, space="PSUM") as ops:
                    t_WoT = ofix.tile([128, 4, D], F32R)
                    t_AoT = ofix.tile([128, 4, 8], F32R)
                    t_BoT = ofix.tile([8, D], F32R)
                    nc.sync.dma_start(out=t_WoT, in_=WoT)
                    nc.sync.dma_start(out=t_AoT, in_=AoT)
                    nc.sync.dma_start(out=t_BoT, in_=BoT)
                    top = ops1.tile([8, QH], F32, tag="top")
                    for hdt in range(4):
                        for nn in range(2):
                            nc.tensor.matmul(
                                top[:, nn * 512:(nn + 1) * 512],
                                t_AoT[:, hdt, :],
                                attnT[:, hdt, nn * 512:(nn + 1) * 512],
                                start=(hdt == 0), stop=(hdt == 3))
                    t_to = op.tile([8, QH], F32R, tag="t_to")
                    nc.scalar.copy(t_to, top)
                    for qt in range(NQT):
                        qsl = slice(qt * 128, (qt + 1) * 128)
                        po = ops.tile([128, D], F32, tag="po")
                        for hdt in range(4):
                            nc.tensor.matmul(po, attnT[:, hdt, qsl],
                                             t_WoT[:, hdt, :],
                                             start=(hdt == 0), stop=False)
                        nc.tensor.matmul(po, t_to[0:8, qsl], t_BoT[0:8, :],
                                         start=False, stop=True)
                        ob = op.tile([128, D], F32, tag="ob")
                        nc.vector.tensor_copy(ob, po)
                        nc.sync.dma_start(out=out_o[qsl, :], in_=ob)

    nc.compile()
    _PROG_CACHE["nc"] = nc
    return nc


def _host_prep(inputs):
    """Build the 8 per-core input maps."""
    q, k, v, mask = inputs["q"], inputs["k"], inputs["v"], inputs["mask"]
    rel_emb = inputs["rel_emb"]

    table = _bucket_table()                        # [4095]
    biasrow = rel_emb[table, :]                    # [4095, H] fp32, exact
    b31 = rel_emb[31, :].astype(np.float64)        # far const, rel >= 128
    b15 = rel_emb[15, :].astype(np.float64)        # far const, rel <= -128

    def cm(x):  # [T, 512] -> [128, 4, T] channel-major contiguous
        return np.ascontiguousarray(
            x.T.reshape(4, 128, -1).transpose(1, 0, 2))

    def wprep(W, s=1.0):
        Wt = np.ascontiguousarray((W * s).T.astype(np.float32))  # [c(in), out]
        return np.ascontiguousarray(Wt.reshape(4, 128, 512).transpose(1, 0, 2))

    WqT = wprep(inputs["Wq"], 1.0 / 8.0)
    WkT = wprep(inputs["Wk"])
    WvT = wprep(inputs["Wv"])
    WoT = wprep(inputs["Wo"])

    def aprep(A):  # [8, 512] -> A.T [512, 8] -> [128, 4, 8]
        At = np.ascontiguousarray(A.T.astype(np.float32))
        return np.ascontiguousarray(At.reshape(4, 128, 8).transpose(1, 0, 2))

    AqT = aprep(inputs["Aq"])
    AvT = aprep(inputs["Av"])
    AoT = aprep(inputs["Ao"])
    BqT = np.ascontiguousarray(inputs["Bq"].T.astype(np.float32)) / 64.0
    BvT = np.ascontiguousarray(inputs["Bv"].T.astype(np.float32)) / 8.0
    BoT = np.ascontiguousarray(inputs["Bo"].T.astype(np.float32)) / 8.0

    p = np.arange(128)[:, None]
    in_maps = []
    for cid in range(NCORES):
        b, half = cid // 2, cid % 2
        r0 = half * QH

        # sliding windows: idx = p - f' + 4095 - r0 into biasrow
        ffl = np.arange(V2W)[None, :] + 128          # f' = ffL + 128
        ffr = np.arange(V2W)[None, :] + 1920         # f' = ffR + 1920
        idxl = np.clip(p - ffl + 4095 - r0, 0, 4094)
        idxr = np.clip(p - ffr + 4095 - r0, 0, 4094)
        bb = b31 if half == 0 else b15               # [H]
        V2La = np.exp(biasrow[idxl, :].astype(np.float64)
                      - b31[None, None, :]).astype(np.float16)
        V2Ra = np.exp(biasrow[idxr, :].astype(np.float64)
                      - b31[None, None, :]).astype(np.float16)
        # -> [128, H, V2W]
        V2La = np.ascontiguousarray(V2La.transpose(0, 2, 1))
        V2Ra = np.ascontiguousarray(V2Ra.transpose(0, 2, 1))
        bcoa = np.ascontiguousarray(
            np.broadcast_to(bb.astype(np.float32)[None, :], (128, H)))
        sfxa = np.ascontiguousarray(np.broadcast_to(
            np.exp(b31 - bb).astype(np.float32)[None, :], (128, H)))

        # position-bias window for head cid
        ffw = np.arange(3968)[None, :]
        W2Oa = np.ascontiguousarray(
            biasrow[np.clip(ffw - p + 127, 0, 4094), cid].astype(np.float32))

        in_maps.append({
            "qT": cm(q[b, r0:r0 + QH, :]),
            "kT": cm(k[b]),
            "vT": cm(v[b]),
            "mk": np.ascontiguousarray(mask[b, r0:r0 + QH, :]),
            "WqT": WqT, "WkT": WkT, "WvT": WvT, "WoT": WoT,
            "AqT": AqT, "AvT": AvT, "AoT": AoT,
            "BqT": BqT, "BvT": BvT, "BoT": BoT,
            "V2L": V2La, "V2R": V2Ra, "bco": bcoa, "sfx": sfxa,
            "W2O": W2Oa,
        })
    return in_maps


def kernel(**inputs):
    nc = _build_program()
    in_maps = _host_prep(inputs)
    res = run_bass_kernel_spmd(nc, in_maps, list(range(NCORES)))

    out = np.empty((B, TQ, D), np.float32)
    pb = np.empty((H, 1, TQ, TKV), np.float32)
    for cid in range(NCORES):
        b, half = cid // 2, cid % 2
        r0 = half * QH
        out[b, r0:r0 + QH, :] = res.results[cid]["out_o"]
        pb[cid, 0] = res.results[cid]["pb_o"]
    return out, pb
